# revision 84
# baseline (speedup 1.0000x reference)
"""Trainium2 Bass kernel for MultiHeadGeometryAttention.

Math (per batch b):
  q = x @ Wq + bq ; k = keys @ Wk + bk ; v = values @ Wv + bv   (per-head d=64)
  S_h = q_h k_h^T / 8
  w = softmax(log(clip(g,1e-6)) + where(mask, -inf, S))
    = g_eff * exp(S/8 - c) / rowsum(...)   with g_eff = where(mask, 0, clip(g,1e-6))
  out = (w @ v) reshaped @ Wo + bo ; y = LayerNorm(x + out) * gamma + beta

Sharding: 8 cores = 4 batches x 2 query-halves (512 q rows per core).
No collectives; K/V projections duplicated within a batch pair.

Numerics: every matmul except the score PSUM accumulation and the output
projection runs in fp8(e4m3) DoubleRow mode (two contraction tiles per PE
pass).  For the scores the d=64 contraction is packed as [32 partitions x
2 slots]; the K/Q projection weights are column-permuted on the host so
the projections write that layout directly.  exp(S/8-3) and the geometry
gate are fp8 so the PV matmul is DoubleRow over key-tile pairs; the -3
logit shift keeps exp() inside fp8 range and cancels in the softmax
normalization.  The output projection runs bf16; residual + LayerNorm f32.

All DRAM inputs are pre-packed on the host so every SBUF tile load is a
large contiguous block; input loads are hoisted so DMA runs ahead of
compute.  The attention loop is software-pipelined (S/exp/gate run AHEAD
iterations ahead of the PV accumulation so the PE never head-of-line
blocks).
"""

import numpy as np
from contextlib import ExitStack

import concourse.bass as bass
import concourse.bacc as bacc
import concourse.tile as tile
from concourse import mybir
from concourse.bass_utils import run_bass_kernel_spmd

P = 128
B, NQ, NK, D, H, DK, DV = 4, 1024, 1024, 1024, 16, 64, 64
NQL = 512           # q rows per core
NCORES = 8
EXPS = 0.125        # exp scale: logits = S/8
EXPB = -3.0         # logit shift: keeps fp8 exp() in range; cancels in softmax
LN_EPS = 1e-5

F32 = mybir.dt.float32
BF16 = mybir.dt.bfloat16
FP8 = mybir.dt.float8e4
DR = mybir.MatmulPerfMode.DoubleRow

DT_KT = 8           # D // P contraction tiles
HT = H * DK // P    # 8 head-dim tiles of 128
G4 = H // 4         # 4 groups of 4 heads (for the packed score layout)
KT_TILES = NK // P  # 8 key tiles
MT = KT_TILES // 2  # 4 key-tile pairs
QT_TILES = NQL // P # 4 query tiles
AHEAD = 3           # attention software-pipeline depth
EP_DELAY = 2        # iterations between a pair's last PV and its epilogue


def build_nc(with_bias=True, with_affine=True):
    nc = bacc.Bacc(None, target_bir_lowering=False)

    # host-packed layouts: every tile load below is contiguous in DRAM
    xqT = nc.dram_tensor("xqT", [P, DT_KT, NQL], FP8, kind="ExternalInput")
    keysT = nc.dram_tensor("keysT", [P, DT_KT, NK], FP8, kind="ExternalInput")
    valuesT = nc.dram_tensor("valuesT", [P, DT_KT, NK], FP8, kind="ExternalInput")
    xq = nc.dram_tensor("xq", [NQL, D], F32, kind="ExternalInput")
    # g gate: [t][j][p][m][slot][q]; one DMA per (t, j) covers 4 kt-pairs
    g_t = nc.dram_tensor("g_t", [HT, 2, P, MT, 2, NQL], FP8,
                         kind="ExternalInput")
    wq = nc.dram_tensor("wq", [P, DT_KT, D], FP8, kind="ExternalInput")
    wk = nc.dram_tensor("wk", [P, DT_KT, D], FP8, kind="ExternalInput")
    wv = nc.dram_tensor("wv", [P, DT_KT, D], FP8, kind="ExternalInput")
    wo = nc.dram_tensor("wo", [P, HT, D], BF16, kind="ExternalInput")
    ones_bf = nc.dram_tensor("ones_bf", [P, H], FP8, kind="ExternalInput")
    expb_in = nc.dram_tensor("expb_in", [P, 1], F32, kind="ExternalInput")
    e16_in = nc.dram_tensor("e16_in", [H, H * 64], BF16, kind="ExternalInput")
    gamma2d = nc.dram_tensor("gamma2d", [P, D], F32, kind="ExternalInput")
    beta2d = nc.dram_tensor("beta2d", [P, D], F32, kind="ExternalInput")
    if with_bias:
        bq = nc.dram_tensor("bq_s", [1, D], FP8, kind="ExternalInput")
        bk = nc.dram_tensor("bk_in", [1, D], FP8, kind="ExternalInput")
        bv = nc.dram_tensor("bv_in", [1, D], FP8, kind="ExternalInput")
        bo = nc.dram_tensor("bo_in", [1, D], BF16, kind="ExternalInput")
        ones_row_d = nc.dram_tensor("ones_row_d", [1, D], FP8,
                                    kind="ExternalInput")
        ones_row_b = nc.dram_tensor("ones_row_b", [1, D], BF16,
                                    kind="ExternalInput")
    y = nc.dram_tensor("y", [NQL, D], F32, kind="ExternalOutput")

    with tile.TileContext(nc) as tc, ExitStack() as ctx:
        persist = ctx.enter_context(tc.tile_pool(name="persist", bufs=1))

        # ---- constants + all input loads, hoisted so DMA runs early ----
        ones_hb = persist.tile([P, H], FP8, tag="ones_hb")
        nc.sync.dma_start(out=ones_hb, in_=ones_bf[:, :])
        expb_sb = persist.tile([P, 1], F32, tag="expb_sb")
        nc.sync.dma_start(out=expb_sb, in_=expb_in[:, :])
        e16_sb = persist.tile([H, H * 64], BF16, tag="e16_sb")
        nc.gpsimd.dma_start(out=e16_sb, in_=e16_in[:, :])
        if with_bias:
            ones_rowf = persist.tile([1, D], FP8, tag="ones_rowf")
            nc.sync.dma_start(out=ones_rowf, in_=ones_row_d[0:1, :])
            ones_rowb = persist.tile([1, D], BF16, tag="ones_rowb")
            nc.sync.dma_start(out=ones_rowb, in_=ones_row_b[0:1, :])
            bq_sb = persist.tile([1, D], FP8, tag="bq_sb")
            nc.sync.dma_start(out=bq_sb, in_=bq[0:1, :])
            bk_sb = persist.tile([1, D], FP8, tag="bk_sb")
            nc.sync.dma_start(out=bk_sb, in_=bk[0:1, :])
            bv_sb = persist.tile([1, D], FP8, tag="bv_sb")
            nc.sync.dma_start(out=bv_sb, in_=bv[0:1, :])
            bo_sb = persist.tile([1, D], BF16, tag="bo_sb")
            nc.sync.dma_start(out=bo_sb, in_=bo[0:1, :])

        vT_all = persist.tile([P, DT_KT, NK], FP8, name="vT_all")
        wv_all = persist.tile([P, DT_KT, D], FP8, name="wv_all")
        kT_all = persist.tile([P, DT_KT, NK], FP8, name="kT_all")
        wk_all = persist.tile([P, DT_KT, D], FP8, name="wk_all")
        xqT_all = persist.tile([P, DT_KT, NQL], FP8, name="xqT_all")
        wq_all = persist.tile([P, DT_KT, D], FP8, name="wq_all")
        wo_all = persist.tile([P, HT, D], BF16, name="wo_all")
        nc.sync.dma_start(out=vT_all, in_=valuesT[:, :, :])
        nc.sync.dma_start(out=wv_all, in_=wv[:, :, :])
        nc.scalar.dma_start(out=kT_all, in_=keysT[:, :, :])
        nc.scalar.dma_start(out=wk_all, in_=wk[:, :, :])
        nc.gpsimd.dma_start(out=xqT_all, in_=xqT[:, :, :])
        nc.gpsimd.dma_start(out=wq_all, in_=wq[:, :, :])
        nc.gpsimd.dma_start(out=wo_all, in_=wo[:, :, :])
        wo_sb = [wo_all[:, i, :] for i in range(HT)]
        gamma_b = persist.tile([P, D], F32, tag="gamma_b")
        nc.gpsimd.dma_start(out=gamma_b, in_=gamma2d[:, :])
        beta_b = persist.tile([P, D], F32, tag="beta_b")
        nc.gpsimd.dma_start(out=beta_b, in_=beta2d[:, :])

        # ---- persistent activations ----
        # Vaug pairs: [m][128, 2, H*65] fp8 (per head: 64 v cols + ones col)
        vaug = [persist.tile([P, 2, H * 65], FP8, tag=f"vaug{m}",
                             name=f"vaug{m}") for m in range(MT)]
        # packed scores layout: per 4-head group g, [32a:32a+32) holds head
        # 4g+a with d = 32*slot + (p-32a)
        kt8 = [persist.tile([P, 2, NK], FP8, tag=f"kt8{g}", name=f"kt8{g}")
               for g in range(G4)]
        qt8 = [persist.tile([P, 2, NQL], FP8, tag=f"qt8{g}", name=f"qt8{g}")
               for g in range(G4)]
        # base partition 96 is not addressable by engines: head a=3 of each
        # group lives in a hopped-down copy at base 0
        ktb = [persist.tile([32, 2, NK], FP8, tag=f"ktb{g}", name=f"ktb{g}")
               for g in range(G4)]
        qtb = [persist.tile([32, 2, NQL], FP8, tag=f"qtb{g}", name=f"qtb{g}")
               for g in range(G4)]
        ot_sb = [persist.tile([P, NQL], BF16, tag=f"ot{i}", name=f"ot{i}")
                 for i in range(HT)]
        # raw per-head PV outputs (row 64 = rowsum), staged to SBUF so the
        # softmax divide runs after the attention loop, off its critical path
        osb = [persist.tile([65, NQL], BF16, tag=f"osb{h}", name=f"osb{h}")
               for h in range(H)]
        r16 = persist.tile([H, NQL], F32, tag="r16")

        def proj_group(ps, stat_all, stat_cols, mov_all, mov_cols,
                       bias_lhsT, bias_rhs):
            """ps = sum_dt stat[:,dt,stat_cols]^T mov[:,dt,mov_cols] (+bias),
            fp8 DoubleRow over pairs of contraction tiles."""
            for dp in range(DT_KT // 2):
                last = (dp == DT_KT // 2 - 1) and not with_bias
                nc.tensor.matmul(
                    ps,
                    lhsT=stat_all[:, 2 * dp:2 * dp + 2, stat_cols],
                    rhs=mov_all[:, 2 * dp:2 * dp + 2, mov_cols],
                    start=(dp == 0), stop=last, perf_mode=DR)
            if with_bias:
                nc.tensor.matmul(ps, lhsT=bias_lhsT, rhs=bias_rhs,
                                 start=False, stop=True)

        # ================= Phase V: Vaug = values^T-proj =================
        with tc.tile_pool(name="pv_ps", bufs=2, space="PSUM") as pv_ps:
            for i in range(KT_TILES):
                m, slot = i // 2, i % 2
                nc.scalar.copy(
                    out=vaug[m][:, slot, :].rearrange(
                        "p (h c) -> p h c", c=65)[:, :, 64:65],
                    in_=ones_hb[:, :, None])
                ps = pv_ps.tile([P, 2, 512], F32, tag="vps")
                for half in range(2):
                    proj_group(
                        ps[:, half, :], vT_all, slice(i * P, (i + 1) * P),
                        wv_all, slice(half * 512, (half + 1) * 512),
                        None if not with_bias else ones_rowf[0:1, 0:P],
                        None if not with_bias else bv_sb[0:1, half * 512:(half + 1) * 512])
                nc.scalar.copy(
                    out=vaug[m][:, slot, :].rearrange(
                        "p (h c) -> p h c", c=65)[:, :, 0:64],
                    in_=ps.rearrange("p a (h c) -> p (a h) c", c=64))

        # ==== Phases K/Q + A fused: the packed-layout K/Q projection groups
        # (wk/wq host column-permuted so out partition p, group g, slot s =
        # head 4g+p//32, d = 32s+p%32) are interleaved into the attention
        # loop so the PE never idles on softmax dependencies (keeps the
        # tensor-engine p-state ramped).  They share one PSUM pool. ====
        with tc.tile_pool(name="pa_gm", bufs=4) as pa_gm, \
             tc.tile_pool(name="pa_pt", bufs=AHEAD + 2) as pa_pt, \
             tc.tile_pool(name="pa_st", bufs=3, space="PSUM") as pa_st, \
             tc.tile_pool(name="pa_ot", bufs=1, space="PSUM") as pa_ot:

            def emit_kproj(g, s):
                c0 = (2 * g + s) * P
                ps = pa_st.tile([P, 2, 512], F32, tag="stps")
                for half in range(2):
                    proj_group(
                        ps[:, half, :], wk_all, slice(c0, c0 + P),
                        kT_all, slice(half * 512, (half + 1) * 512),
                        None if not with_bias else bk_sb[0:1, c0:c0 + P],
                        None if not with_bias else ones_rowf[0:1, 0:512])
                nc.scalar.copy(
                    out=kt8[g][:, s, :],
                    in_=ps.rearrange("p a b -> p (a b)"))
                if s == 1:
                    nc.gpsimd.dma_start(out=ktb[g], in_=kt8[g][96:128, :, :])

            def emit_qproj(g):
                ps = pa_st.tile([P, 2, NQL], F32, tag="stps")
                for s in range(2):
                    c0 = (2 * g + s) * P
                    proj_group(
                        ps[:, s, :], wq_all, slice(c0, c0 + P),
                        xqT_all, slice(0, NQL),
                        None if not with_bias else bq_sb[0:1, c0:c0 + P],
                        None if not with_bias else ones_rowf[0:1, 0:NQL])
                nc.scalar.copy(out=qt8[g], in_=ps)
                nc.gpsimd.dma_start(out=qtb[g], in_=qt8[g][96:128, :, :])

            # prelude: group 0 projections; later groups interleave into
            # the attention iteration stream below
            emit_kproj(0, 0)
            emit_kproj(0, 1)
            emit_qproj(0)
            interleave = {}
            for g in range(1, G4):
                base = (2 * g - 2) * 2 * MT  # attention iter where pair 2(g-1) starts
                interleave[base + 1] = lambda g=g: emit_kproj(g, 0)
                interleave[base + 6] = lambda g=g: emit_kproj(g, 1)
                interleave[base + 11] = lambda g=g: emit_qproj(g)

            seq = [(t, j, m) for t in range(HT) for j in range(2)
                   for m in range(MT)]
            otp = {}
            gts = {}
            pts = {}
            pend_ep = []  # [t, emit_at_index]

            def emit_front(i):
                t, j, m = seq[i]
                h = 2 * t + j
                g, a = h // 4, h % 4
                if j == 0 and m == 0:
                    otp[t] = [pa_ot.tile([65, NQL], F32, tag=f"otp_{jj}",
                                         name=f"otp{t}_{jj}")
                              for jj in range(2)]
                if m == 0:
                    gt = pa_gm.tile([P, MT, 2, NQL], FP8, tag="gt")
                    nc.gpsimd.dma_start(out=gt, in_=g_t[t, j])
                    gts[(t, j)] = gt
                # two DoubleRow S matmuls (kt = 2m, 2m+1), packed d=[32x2]
                stp = pa_st.tile([P, 2, NQL], F32, tag="stps")
                for s in range(2):
                    kt = 2 * m + s
                    if a == 3:
                        kl = ktb[g][:, :, kt * P:(kt + 1) * P]
                        qr = qtb[g][:, :, :]
                    else:
                        kl = kt8[g][32 * a:32 * a + 32, :, kt * P:(kt + 1) * P]
                        qr = qt8[g][32 * a:32 * a + 32, :, :]
                    nc.tensor.matmul(
                        stp[:, s, :], lhsT=kl, rhs=qr,
                        start=True, stop=True, perf_mode=DR)
                pt0 = pa_pt.tile([P, 2, NQL], FP8, tag="pt0")
                nc.scalar.activation(
                    out=pt0, in_=stp,
                    func=mybir.ActivationFunctionType.Exp,
                    scale=EXPS, bias=expb_sb[:, 0:1])
                pt = pa_pt.tile([P, 2, NQL], FP8, tag="pt")
                nc.vector.tensor_tensor(
                    out=pt, in0=pt0, in1=gts[(t, j)][:, m, :, :],
                    op=mybir.AluOpType.mult)
                pts[(t, j, m)] = pt

            def emit_pv(i):
                t, j, m = seq[i]
                h = 2 * t + j
                nc.tensor.matmul(
                    otp[t][j],
                    lhsT=vaug[m][:, :, h * 65:(h + 1) * 65],
                    rhs=pts.pop((t, j, m)),
                    start=(m == 0), stop=(m == MT - 1), perf_mode=DR)
                if m == MT - 1:
                    # stage raw head output (+rowsum) to SBUF; divide later.
                    # On DVE: the ACT queue must stay clear for the exps.
                    nc.vector.tensor_scalar(
                        out=osb[h], in0=otp[t][j], scalar1=1.0, scalar2=None,
                        op0=mybir.AluOpType.mult)
                    # gather this head's rowsum row now (pool is idle here)
                    nc.gpsimd.dma_start(out=r16[h:h + 1, :],
                                        in_=osb[h][64:65, :])

            n = len(seq)
            for i in range(n + AHEAD):
                if i in interleave:
                    interleave.pop(i)()
                if i < n:
                    emit_front(i)
                if i >= AHEAD:
                    emit_pv(i - AHEAD)

        # ===== softmax divide epilogue fused with the output projection:
        # Y's accumulation step ht=t needs only pair t's divide, so the Y
        # matmuls for qt 0,1 interleave with the per-pair divides.  Each
        # sweep's residual-add (the PSUM reader) is emitted BEFORE the next
        # sweep reuses its PSUM tag, so the start=True zeroing can't race
        # the read. =====
        with tc.tile_pool(name="pe_ep", bufs=2) as pe_ep, \
             tc.tile_pool(name="pe_r", bufs=1) as pe_r, \
             tc.tile_pool(name="pe_rb", bufs=2, space="PSUM") as pe_rb, \
             tc.tile_pool(name="py_x", bufs=2) as py_x, \
             tc.tile_pool(name="py_t", bufs=2) as py_t, \
             tc.tile_pool(name="py_s", bufs=4) as py_s, \
             tc.tile_pool(name="py_ps", bufs=1, space="PSUM") as py_ps:
            # 1/r = exp(-ln r) on ACT: ~2.5x faster than the DVE
            # iterative reciprocal and emits bf16 directly
            lnr = pe_r.tile([H, NQL], F32, tag="lnr")
            nc.scalar.activation(out=lnr, in_=r16,
                                 func=mybir.ActivationFunctionType.Ln)
            rinv16 = pe_r.tile([H, NQL], BF16, tag="rinv16")
            nc.scalar.activation(out=rinv16, in_=lnr,
                                 func=mybir.ActivationFunctionType.Exp,
                                 scale=-1.0)

            yps = {}

            def y_start(qt):
                xres = py_x.tile([P, D], F32, tag="xres")
                nc.sync.dma_start(out=xres, in_=xq[qt * P:(qt + 1) * P, :])
                yps[qt] = (py_ps.tile([P, D], F32, tag=f"yps{qt % 3}",
                                      name=f"yps{qt}"), xres)

            def y_step(qt, ht):
                ps = yps[qt][0]
                for half in range(2):
                    last = (ht == HT - 1) and not with_bias
                    nc.tensor.matmul(
                        ps[:, half * 512:(half + 1) * 512],
                        lhsT=ot_sb[ht][:, qt * P:(qt + 1) * P],
                        rhs=wo_sb[ht][:, half * 512:(half + 1) * 512],
                        start=(ht == 0), stop=last)
                    if with_bias and ht == HT - 1:
                        nc.tensor.matmul(
                            ps[:, half * 512:(half + 1) * 512],
                            lhsT=ones_rowb[0:1, 0:P],
                            rhs=bo_sb[0:1, half * 512:(half + 1) * 512],
                            start=False, stop=True)

            def divide_pair(t):
                for j in range(2):
                    h = 2 * t + j
                    rb = pe_rb.tile([64, NQL], F32, tag="rb")
                    nc.tensor.matmul(
                        rb, lhsT=e16_sb[:, h * 64:(h + 1) * 64], rhs=rinv16,
                        start=True, stop=True)
                    if j == 0:
                        nc.vector.tensor_tensor(
                            out=ot_sb[t][0:64, :], in0=osb[h][0:64, :],
                            in1=rb, op=mybir.AluOpType.mult)
                    else:
                        tmp = pe_ep.tile([64, NQL], BF16, tag="ottmp")
                        nc.vector.tensor_tensor(
                            out=tmp, in0=osb[h][0:64, :], in1=rb,
                            op=mybir.AluOpType.mult)
                        # partition shift 0-63 -> 64-127 needs a DMA hop
                        nc.gpsimd.dma_start(out=ot_sb[t][64:128, :], in_=tmp)

            def y_finish(qt):
                ps, xres = yps.pop(qt)
                # residual add (psum + sbuf -> sbuf)
                x_t = py_t.tile([P, D], F32, tag="x_t")
                nc.vector.tensor_tensor(
                    out=x_t, in0=ps, in1=xres, op=mybir.AluOpType.add)
                # mean/var in one pass via bn_stats/bn_aggr
                nsub = D // nc.vector.BN_STATS_FMAX
                stats = py_s.tile([P, nsub, nc.vector.BN_STATS_DIM], F32,
                                  tag="stats")
                xg = x_t.rearrange("p (s f) -> p s f", s=nsub)
                for s in range(nsub):
                    nc.vector.bn_stats(out=stats[:, s, :], in_=xg[:, s, :])
                mv = py_s.tile([P, nc.vector.BN_AGGR_DIM], F32, tag="mv")
                nc.vector.bn_aggr(out=mv, in_=stats)
                var_eps = py_s.tile([P, 1], F32, tag="var_eps")
                nc.vector.tensor_scalar(
                    out=var_eps, in0=mv[:, 1:2], scalar1=LN_EPS, scalar2=None,
                    op0=mybir.AluOpType.add)
                rvar = py_s.tile([P, 1], F32, tag="rvar")
                nc.vector.reciprocal(out=rvar, in_=var_eps)
                rstd = py_s.tile([P, 1], F32, tag="rstd")
                nc.scalar.sqrt(out=rstd, in_=rvar)
                xhat = py_t.tile([P, D], F32, tag="xhat")
                nc.vector.tensor_scalar(
                    out=xhat, in0=x_t, scalar1=mv[:, 0:1], scalar2=rstd,
                    op0=mybir.AluOpType.subtract, op1=mybir.AluOpType.mult)
                if with_affine:
                    yout = py_t.tile([P, D], F32, tag="yout")
                    nc.vector.tensor_tensor(
                        out=yout, in0=xhat, in1=gamma_b,
                        op=mybir.AluOpType.mult)
                    nc.vector.tensor_tensor(
                        out=yout, in0=yout, in1=beta_b, op=mybir.AluOpType.add)
                else:
                    yout = xhat
                nc.sync.dma_start(out=y[qt * P:(qt + 1) * P, :], in_=yout)

            # sweep A: divides interleaved with Y accumulation for qt 0-2
            # (3 accumulators x 2 banks + 2 rb banks = all 8 PSUM banks)
            y_start(0)
            y_start(1)
            y_start(2)
            for t in range(HT):
                divide_pair(t)
                y_step(0, t)
                y_step(1, t)
                y_step(2, t)
            y_finish(0)
            y_finish(1)
            y_finish(2)
            # sweep B: just qt 3 (its tag safely released by y_finish(0))
            y_start(3)
            for t in range(HT):
                y_step(3, t)
            y_finish(3)

    nc.compile()
    return nc


_NC_CACHE = {}


def _get_nc(with_bias, with_affine):
    key = (with_bias, with_affine)
    if key not in _NC_CACHE:
        _NC_CACHE[key] = build_nc(with_bias, with_affine)
    return _NC_CACHE[key]


def _kq_perm():
    """Column permutation for Wk/Wq: packed column c=(2g+s)*128+p holds
    hd index (4g + p//32)*64 + 32*s + (p%32)."""
    c = np.arange(H * DK)
    g, r = c // 256, c % 256
    s, p = r // 128, r % 128
    return (4 * g + p // 32) * 64 + 32 * s + (p % 32)


def make_in_maps(queries, keys, values, geometry, attention_mask,
                 Wq, bq, Wk, bk, Wv, bv, Wo, bo, ln_gamma, ln_beta,
                 with_bias, with_affine):
    bf16 = mybir.dt.np(BF16)
    fp8 = mybir.dt.np(FP8)
    f32 = np.float32
    perm = _kq_perm()

    def pack_w(w, dt):
        # [D, D] -> [P, DT_KT, D]: tile rows by 128, partition-major
        return np.ascontiguousarray(
            np.asarray(w, dtype=f32).reshape(DT_KT, P, D).transpose(1, 0, 2)
        ).astype(dt)

    shared = {
        "wq": pack_w(np.asarray(Wq, dtype=f32)[:, perm], fp8),
        "wk": pack_w(np.asarray(Wk, dtype=f32)[:, perm], fp8),
        "wv": pack_w(Wv, fp8),
        "wo": pack_w(Wo, bf16),
        "ones_bf": np.ones((P, H), dtype=fp8),
        "expb_in": np.full((P, 1), EXPB, dtype=f32),
        "e16_in": np.kron(np.eye(H, dtype=f32),
                          np.ones((1, 64), dtype=f32)).astype(bf16),
        "gamma2d": np.broadcast_to(np.asarray(ln_gamma, dtype=f32), (P, D)).copy(),
        "beta2d": np.broadcast_to(np.asarray(ln_beta, dtype=f32), (P, D)).copy(),
    }
    if with_bias:
        for nm, bb, dt in (("bq_s", np.asarray(bq, dtype=f32)[perm], fp8),
                           ("bk_in", np.asarray(bk, dtype=f32)[perm], fp8),
                           ("bv_in", bv, fp8), ("bo_in", bo, bf16)):
            shared[nm] = np.asarray(bb, dtype=f32).astype(dt).reshape(1, D)
        shared["ones_row_d"] = np.ones((1, D), dtype=fp8)
        shared["ones_row_b"] = np.ones((1, D), dtype=bf16)

    # g_eff = where(mask, 0, clip(g, 1e-6)): fold the -inf mask into the
    # geometry gate (exactly equivalent post-softmax)
    g_eff = np.where(attention_mask, np.float32(0),
                     np.clip(geometry, 1e-6, None).astype(f32))  # [B,H,NQ,NK]

    def packT(x):  # [N, D] -> [P, DT_KT, cols]: one contiguous DMA
        return np.ascontiguousarray(
            np.asarray(x, dtype=f32).T.reshape(DT_KT, P, -1)
            .transpose(1, 0, 2)).astype(fp8)

    in_maps = []
    for c in range(NCORES):
        b, qh = c // 2, c % 2
        qs = slice(qh * NQL, (qh + 1) * NQL)
        # [H, NQL, NK] -> [H=2t+j, NK=256m+128s+p, NQL] -> [t,j,p,m,s,q]
        g_slice = g_eff[b, :, qs, :].transpose(0, 2, 1).reshape(
            HT, 2, MT, 2, P, NQL).transpose(0, 1, 4, 2, 3, 5)
        in_maps.append({
            "xqT": packT(queries[b, qs]),
            "keysT": packT(keys[b]),
            "valuesT": packT(values[b]),
            "xq": np.ascontiguousarray(queries[b, qs], dtype=f32),
            "g_t": np.ascontiguousarray(g_slice.astype(fp8)),
            **shared,
        })
    return in_maps


def kernel(queries, keys, values, geometry, attention_mask,
           Wq, bq, Wk, bk, Wv, bv, Wo, bo, ln_gamma, ln_beta, **run_kwargs):
    with_bias = bool(np.any(bq) or np.any(bk) or np.any(bv) or np.any(bo))
    with_affine = bool(np.any(np.asarray(ln_gamma) != 1) or np.any(ln_beta))
    nc = _get_nc(with_bias, with_affine)
    in_maps = make_in_maps(queries, keys, values, geometry, attention_mask,
                           Wq, bq, Wk, bk, Wv, bv, Wo, bo, ln_gamma, ln_beta,
                           with_bias, with_affine)
    res = run_bass_kernel_spmd(nc, in_maps, core_ids=list(range(NCORES)),
                               **run_kwargs)
    out = np.empty((B, NQ, D), np.float32)
    for c in range(NCORES):
        b, qh = c // 2, c % 2
        out[b, qh * NQL:(qh + 1) * NQL, :] = res.results[c]["y"]
    if run_kwargs:
        kernel.last_results = res
    return out
